# revision 3
# baseline (speedup 1.0000x reference)
"""Tropical max-plus 2D conv (BroadcastConv tropical_max) on 8 Trainium2 cores.

out[b,o,y,x] = max_{c,i,j} img_pad[b,c,y+i,x+j] + kflip[o,c,i,j]
  imgs [4,32,128,128] f32, kernel [32,32,5,5] f32, stride=1, pad=2, dil=1.

Sharding: output channels O=32 split across 8 cores (OL=4 per core).

Strategy (all bf16 on device; harness gate rel_err < 2e-2, bf16 adds ~0.4%):
- partitions p = c_l*32 + b*8 + yhi (c_l: input-channel quadrant, b: batch,
  yhi: y/16). Input channels live in partition quadrants, so tile loads carry
  4 distinct channels each - no o-replication, only 5.4MB DMA per core.
- free dim = flat (ylo:16, xcol:132) = 2112. One SBUF tile [128, 2646] per
  channel-group holds 20 padded image rows flattened; every kernel tap (i,j)
  is a flat free-dim offset i*132+j into it - no per-tap DMA.
- Each (channel-group, tap, o_l) update is ONE custom DVE instruction
  (MAX_ADD_ANT): acc[o_l] = max(tile[off:off+2112] + k[o,c,i,j], acc[o_l])
  with k as a per-partition [128,1] f32 scalar. The op carries a
  hand-authored 2x_1p uop program, so at bf16 it streams 2 elem/cycle/lane
  (stock scalar_tensor_tensor has no 2x uop and runs 1x; that is the whole
  speedup). 800 ops/core, ~1.26us each.
- The per-partition acc holds a partial max over that quadrant's 8 channels;
  the final 4-way quadrant max runs on the host after the output DMA.
"""

import numpy as np
from ml_dtypes import bfloat16

NCORES = 8
B, C, H, W = 4, 32, 128, 128
O, KH, KW = 32, 5, 5
OL = O // NCORES
PAD = 2
PH = H + 2 * PAD  # 132
PW = W + 2 * PAD  # 132
YHI, YLO = 8, 16
FD = YLO * W      # 2048 (2-D APs, pad cols dropped)
TLEN = 2646       # 20*132 + 6 tail, even
CSUB = C // 4     # 8 channel-groups, 4 channels (quadrants) each
NK = CSUB * KH * KW * OL  # 800 ops
NEG = float("-inf")
NTBUF = 3         # rotating tile buffers (DMA overlaps compute)

_CACHE = {}

# ---------------------------------------------------------------------------
# Custom DVE op: out = max(in0 + s0, in1), with a hand-written 2x_1p program.
# ---------------------------------------------------------------------------

_OP_NAME = "MAX_ADD_ANT"


def _op_reference(in0, in1, s0, s1, imm2):
    s = s0
    if isinstance(s, np.ndarray) and in0.ndim > 2:
        s = s.reshape(s.shape[0], *([1] * (in0.ndim - 1)))
    return np.maximum(in0 + s, np.asarray(in1).reshape(in0.shape))


def _build_2x_uop():
    from concourse.dve_uop import (
        UopConfig, UopDpConfig, InpSel, OutSel, OutPath, AluOp, AluInp,
        DelayInp, Trigger,
    )

    P, A = DelayInp.PREV_DELAY, DelayInp.PREV_ALU_OUT

    def dp(op=AluOp.BYPASS, s0=AluInp.PREV_ALU_OUT, s1=AluInp.PREV_ALU_OUT,
           delay=None, den=None):
        return UopDpConfig(
            op=op, alu_src0=s0, alu_src1=s1,
            delay=list(delay) if delay else [P] * 7,
            alu_out_enable=1, swap_enable=0, alu_out_a_enable=0,
            alu_out_b_enable=0,
            delay_enable=list(den) if den else [0] * 7,
            idx0_sel=0, idx1_sel=0,
        )

    # 2x_1p: slots 0=SRC_0 (feeds stage0 via PREV_ALU_OUT), 1=SRC_1,
    # 2=SRC_0_HI, 3=SRC_1_HI, 4=CONST_0. Lanes at stage0: L0=SRC_1,
    # L1=SRC_0_HI, L2=SRC_1_HI, L3=CONST_0.  Elem0 result ends on lane0
    # (-> WR0_LO), elem1 result in the ALU chain (-> WR0_HI).
    stages = [
        dp(AluOp.ADD, AluInp.PREV_ALU_OUT, AluInp.PREV_DELAY_3,
           den=[1, 1, 1, 1, 0, 0, 0]),                      # t0 = x0 + C0
        dp(AluOp.ADD, AluInp.PREV_DELAY_1, AluInp.PREV_DELAY_3,
           delay=[P, A, P, P, P, P, P], den=[1, 1, 1, 0, 0, 0, 0]),
        dp(AluOp.MAX, AluInp.PREV_DELAY_1, AluInp.PREV_DELAY_0,
           delay=[A, P, P, P, P, P, P], den=[1, 0, 1, 0, 0, 0, 0]),
        dp(AluOp.MAX, AluInp.PREV_DELAY_0, AluInp.PREV_DELAY_2,
           delay=[A, P, P, P, P, P, P], den=[1, 0, 0, 0, 0, 0, 0]),
        dp(den=[1, 0, 0, 0, 0, 0, 0]),
        dp(den=[1, 0, 0, 0, 0, 0, 0]),
        dp(den=[1, 0, 0, 0, 0, 0, 0]),
        dp(den=[1, 0, 0, 0, 0, 0, 0]),
    ]
    return UopConfig(
        inp=[InpSel.SRC_0, InpSel.SRC_1, InpSel.SRC_0_HI, InpSel.SRC_1_HI,
             InpSel.CONST_0, InpSel.ZERO, InpSel.ZERO, InpSel.ZERO],
        inp_enable=[1, 1, 1, 1, 1, 0, 0, 0],
        out={OutPath.WR0_LO: OutSel.DELAY_0, OutPath.WR0_HI: OutSel.ALU_OUT,
             OutPath.WR1_LO: OutSel.ALU_OUT, OutPath.WR1_HI: OutSel.ALU_OUT},
        out_enable={OutPath.WR0_LO: 1, OutPath.WR0_HI: 1,
                    OutPath.WR1_LO: 0, OutPath.WR1_HI: 0},
        require_inp0=1, require_inp1=1,
        trigger=(Trigger.SRC_TENSOR_DONE, Trigger.NONE, Trigger.NONE),
        next_uop=(0, 0, 0),
        datapath_config=stages,
    )


def _register_op():
    from concourse import dve_ops
    from concourse.dve_ops import DveOp, OPS, CUSTOM_DVE_SPECS
    from concourse.dve_spec import Spec, Src0, Src1, C0, maxx, lower
    from concourse.dve_uop import DveOpSpec

    if any(op.name == _OP_NAME for op in OPS):
        return
    spec = Spec(body=maxx(Src0 + C0, Src1), reference=_op_reference)
    op = DveOp(_OP_NAME, spec, subdim=False, uops_sha={})
    OPS.append(op)
    CUSTOM_DVE_SPECS[_OP_NAME] = spec
    dve_ops._SUB_OPCODE_FOR_NAME[_OP_NAME] = (
        dve_ops._CUSTOM_DVE_ROW_BASE + len(OPS) - 1
    )
    row = dve_ops.get_dve_sub_opcode(_OP_NAME)
    u2x = _build_2x_uop()
    u2x.validate("v3")
    compiled = DveOpSpec(
        name=_OP_NAME, opcode=row, uops=lower(spec, ver="v3"),
        uops_2x=[u2x], perf_max=1, rd1_en=True,
    )
    compiled.validate("v3")
    dve_ops._COMPILE_CACHE[(_OP_NAME, "v3")] = compiled


def _emit_max_add(nc, out, in0, in1, scalar):
    """acc(out) = max(in0 + scalar, in1); perf_max=1 selects the 2x slot."""
    import concourse.mybir as mybir
    from concourse import bass_isa
    from concourse.dve_ops import get_dve_sub_opcode

    vec = nc.vector
    if _OP_NAME not in nc.m.ant_custom_dve_ops:
        nc.m.ant_custom_dve_ops = sorted(
            {*nc.m.ant_custom_dve_ops, _OP_NAME})
    ins = [
        vec.lower_ap(in0, for_isa=True, opt=True),
        vec.lower_ap(in1, for_isa=True, opt=True),
        vec.lower_ap(scalar, for_isa=True),
        mybir.ImmediateValue(dtype=mybir.dt.float32, value=0.0),
    ]
    outs = [vec.lower_ap(out, for_isa=True, opt=True)]
    return vec.add_instruction(
        bass_isa.InstCustomDveAnt(
            name=nc.get_next_instruction_name(),
            op_name=_OP_NAME,
            rd1_en=True,
            subdim=0,
            imm2=0.0,
            shape=bass_isa.CustomDveShape.TTSS,
            row=get_dve_sub_opcode(_OP_NAME),
            perf_max=1,
            isa_opcode=nc.isa.Opcode[
                "NEURON_ISA_TPB_OPCODE_CUSTOM_DVE_ANT_0"].value,
            ins=ins,
            outs=outs,
        )
    )


# ---------------------------------------------------------------------------
# Program
# ---------------------------------------------------------------------------

def _op_sequence():
    """(s, tap, o) order: acc index cycles with period 4 AND the tap window
    changes every op (dependency distance 4, no repeated in0 window)."""
    seq = []
    for s in range(CSUB):
        for q in range(OL):
            for r in range(KH * KW):
                seq.append((s, r, (r + q) % OL))
    return seq


def _build_program():
    import concourse.mybir as mybir
    from concourse import bacc
    from concourse.tile import TileContext

    _register_op()
    f32, bf16 = mybir.dt.float32, mybir.dt.bfloat16
    nc = bacc.Bacc("TRN2", target_bir_lowering=False)
    y_d = nc.declare_dram_parameter("imgsr", [CSUB, 128, TLEN], bf16,
                                    isOutput=False)
    k_d = nc.declare_dram_parameter("kprep", [128, NK], f32, isOutput=False)
    out_d = nc.declare_dram_parameter("out", [OL, 128, FD], bf16,
                                      isOutput=True)

    with TileContext(nc) as tc:
        with tc.tile_pool(name="sbuf", bufs=1) as pool:
            ktab = pool.tile([128, NK], f32, tag="ktab", name="ktab")
            accs = [pool.tile([128, FD], bf16, tag=f"acc{a}", name=f"acc{a}")
                    for a in range(OL)]
            tiles = [pool.tile([128, TLEN], bf16, tag=f"T{s}", name=f"T{s}")
                     for s in range(NTBUF)]

            nc.sync.dma_start(out=ktab[:], in_=k_d[:])
            for a in accs:
                nc.vector.memset(a[:], NEG)

            idx = 0
            cur_s = -1
            for s, r, o in _op_sequence():
                if s != cur_s:
                    cur_s = s
                    nc.sync.dma_start(out=tiles[s % NTBUF][:], in_=y_d[s])
                t = tiles[s % NTBUF]
                off = (r // KW) * PW + (r % KW)
                win = t[:, off:off + YLO * PW].rearrange(
                    'p (u x) -> p u x', u=YLO, x=PW)[:, :, 0:W]
                av = accs[o][:].rearrange('p (u x) -> p u x',
                                          u=YLO, x=W)
                _emit_max_add(nc, av, win, av, ktab[:, idx:idx + 1])
                idx += 1

            for o in range(OL):
                nc.sync.dma_start(out=out_d[o], in_=accs[o][:])

    nc.compile()
    return nc


def _get_program():
    if "nc" not in _CACHE:
        _CACHE["nc"] = _build_program()
    return _CACHE["nc"]


def _prep_inputs(imgs, kernel):
    imgs = np.asarray(imgs, dtype=np.float32)
    padded = np.full((C, B, PH, PW), NEG, dtype=np.float32)
    padded[:, :, PAD:PAD + H, PAD:PAD + W] = imgs.transpose(1, 0, 2, 3)
    flat = padded.reshape(C, B, PH * PW)
    y = np.full((C, B, YHI, TLEN), NEG, dtype=np.float32)
    for yhi in range(YHI):
        st = yhi * YLO * PW
        n = min(TLEN, PH * PW - st)
        y[:, :, yhi, :n] = flat[:, :, st:st + n]
    # [c, b, yhi, L] -> [ci_sub, (c_l, b, yhi), L] with c = c_l*8 + ci_sub
    y = y.reshape(4, CSUB, B, YHI, TLEN).transpose(1, 0, 2, 3, 4)
    y_bf = np.ascontiguousarray(y.reshape(CSUB, 128, TLEN)).astype(bfloat16)

    kf = np.asarray(kernel, dtype=np.float32)[:, :, ::-1, ::-1]  # conv flip
    seq = _op_sequence()
    in_maps = []
    for m in range(NCORES):
        sl = kf[OL * m:OL * (m + 1)]  # [OL, C, KH, KW]
        kp = np.empty((4, NK), dtype=np.float32)
        for c_l in range(4):
            blk = sl[:, c_l * CSUB:(c_l + 1) * CSUB].reshape(
                OL, CSUB, KH * KW)
            for col, (s, r, o) in enumerate(seq):
                kp[c_l, col] = blk[o, s, r]
        kprep = np.repeat(kp, 32, axis=0)  # [128, NK]
        in_maps.append({"imgsr": y_bf, "kprep": np.ascontiguousarray(kprep)})
    return in_maps


def run_spmd(imgs, kernel, trace=False):
    from concourse.bass_utils import run_bass_kernel_spmd

    nc = _get_program()
    in_maps = _prep_inputs(imgs, kernel)
    res = run_bass_kernel_spmd(nc, in_maps, list(range(NCORES)), trace=trace)
    full = np.empty((B, O, H, W), dtype=np.float32)
    for m in range(NCORES):
        r = res.results[m]["out"].astype(np.float32)  # [OL, 128, FD]
        r = r.reshape(OL, 4, B, YHI, YLO, W)
        r = r.max(axis=1)  # quadrant (channel-group) reduce
        full[:, OL * m:OL * (m + 1)] = (
            r.transpose(1, 0, 2, 3, 4).reshape(B, OL, H, W)
        )
    return full, res


def kernel(imgs, kernel, stride=1, padding=2, dilation=1, **_ignored):
    assert int(stride) == 1 and int(padding) == 2 and int(dilation) == 1
    assert tuple(imgs.shape) == (B, C, H, W), imgs.shape
    assert tuple(kernel.shape) == (O, C, KH, KW), kernel.shape
    full, _ = run_spmd(imgs, kernel, trace=False)
    return full
